# revision 21
# baseline (speedup 1.0000x reference)
"""FAVOR causal self-attention (Performer) Trainium2 kernel.

Sharding: 8 cores = 2 (batch) x 4 (head groups of 4 heads). Each core
computes qkv for its heads, runs chunked linear attention (L=128), applies
its slice of the output projection, and returns a partial (T, C) output;
partials are summed on the host (+ b_proj broadcast).

Math (validated vs the jax reference in numpy):
  per head, Eq = exp(omega.T@qT) (x1/16), EkT = exp(omega.T@kT) (x1/16),
  fk = exp(-||k||^2/2)/sqrt(m):
    A_T[tj,ti] = sum_mm EkT[mm,tj] Eq[mm,ti], masked tj<=ti, row-scaled by fk
    [num|den](ti,:) += EqT_chunk.T @ [S|Z]  +  A_T_m.T @ [V|1]
    [S|Z] += Ek_hat.T @ [V|1] accumulated in PSUM, Ek_hat = exp(projk)*fk/16
  y = num/den; the 1/16 scales cancel in the division (kept for fp16 range).

Layout tricks:
  - k stored per head as ktsq_h (128,T): rows 0:64 = kT, 64:128 = kT^2, so one
    matmul against the host const [omega|0 ; 0|-0.5] yields [projk | -nsq].
  - v stored as (128, 4*65) with a ones column after each head's 64, so the
    intra and state matmuls take a single (tj,65) moving operand.
"""
import math
import sys

sys.path.insert(0, "/opt/trn_rl_repo")

import numpy as np

import concourse.bass as bass
import concourse.mybir as mybir
from concourse.tile import TileContext

T, C = 1024, 1024
NH, D, M = 16, 64, 128
L = 128           # chunk length
HPC = 4           # heads per core
NT = T // 128     # 8 token tiles
NK = C // 128     # 8 contraction tiles
F32, F16 = mybir.dt.float32, mybir.dt.float16
LN_SCALE = math.log(1.0 / 16.0)       # folded into Eq and Ek exps
NEG_HALF_LN_M = -0.5 * math.log(M)


def _split_waits(nc):
    """Walrus codegen accepts 1 sync wait per instruction (2 on
    EventSemaphore). Tile can emit more; hoist the excess onto
    EventSemaphore instructions inserted immediately before, same engine."""
    for fn in nc.m.functions:
        for bb in fn.blocks:
            insts = bb.instructions
            i = 0
            while i < len(insts):
                inst = insts[i]
                si = inst.sync_info
                if si is None:
                    i += 1
                    continue
                waits = list(si.on_wait or [])
                cap = 2 if isinstance(inst, mybir.InstEventSemaphore) else 1
                if len(waits) <= cap:
                    i += 1
                    continue
                keep, excess = waits[:cap], waits[cap:]
                new_insts = []
                for j in range(0, len(excess), 2):
                    ev = mybir.InstEventSemaphore(
                        name=nc.get_next_instruction_name(),
                        engine=inst.engine,
                        ins=[],
                        outs=[],
                        sync_info=mybir.SyncInfo(
                            on_wait=excess[j:j + 2], on_update=[]),
                    )
                    nc.register_instruction(ev)
                    new_insts.append(ev)
                inst.sync_info = mybir.SyncInfo(
                    on_wait=keep, on_update=list(si.on_update or []))
                for k, ev in enumerate(new_insts):
                    insts.insert(i + k, ev)
                i += len(new_insts) + 1


def build_bass():
    nc = bass.Bass()

    xT = nc.dram_tensor("xT", [C, T], F16, kind="ExternalInput")
    wqk = nc.dram_tensor("wqk", [C, 4 * 128], F16, kind="ExternalInput")
    wv = nc.dram_tensor("wv", [C, HPC * D], F16, kind="ExternalInput")
    wp = nc.dram_tensor("wp", [HPC * D, C], F16, kind="ExternalInput")
    consts16 = nc.dram_tensor("consts16", [128, 769], F16, kind="ExternalInput")
    consts32 = nc.dram_tensor("consts32", [128, 4], F32, kind="ExternalInput")
    outp = nc.dram_tensor("outp", [T, C], F32, kind="ExternalOutput")

    Exp = mybir.ActivationFunctionType.Exp
    Ident = mybir.ActivationFunctionType.Identity

    with TileContext(nc) as tc:
        with (
            tc.tile_pool(name="big", bufs=1) as big,          # resident data
            tc.tile_pool(name="cpy", bufs=3) as cpy,          # staging tiles
            tc.tile_pool(name="chk", bufs=2) as chk,          # chunk tiles
            tc.tile_pool(name="col", bufs=4) as col,          # small columns
            tc.tile_pool(name="ps", bufs=1, space="PSUM") as ps,
        ):
            # PSUM budget (8 banks): bankA x2, pk x1, pA x1, pYt x2, psS x2.
            def bankA():
                return ps.tile([128, 512], F32, name="bankA", bufs=2)

            # ---- load resident inputs (2 packed const DMAs first) ----
            c16 = big.tile([128, 769], F16, name="c16")
            nc.sync.dma_start(out=c16, in_=consts16[:, :])
            c32 = big.tile([128, 4], F32, name="c32")
            nc.sync.dma_start(out=c32, in_=consts32[:, :])
            om_sb = c16[:, 0:128]
            on_sb = c16[:, 128:257]
            mk_sb = c16[:, 257:385]
            id_sb = c16[:, 385:513]
            bv_sb = c16[0:1, 513:513 + HPC * D]
            bqk_sb = [c32[:, mi:mi + 1] for mi in range(4)]
            ones_r = big.tile([1, 128], F16, name="ones_r")
            nc.vector.memset(ones_r, 1.0)
            lnsc_sb = big.tile([128, 1], F32, name="lnsc")
            nc.vector.memset(lnsc_sb, LN_SCALE)
            nhm_sb = big.tile([128, 1], F32, name="nhm")
            nc.vector.memset(nhm_sb, NEG_HALF_LN_M)

            wqkall = big.tile([128, NK * 512], F16, name="wqkall")
            xtall = big.tile([128, NK * T], F16, name="xtall")
            # k-head weight columns first (first qkv groups), then q
            nc.sync.dma_start(
                out=wqkall[:, :].rearrange("p (a n) -> p a n", a=NK)
                [:, :, 256:512],
                in_=wqk[:, 256:512].rearrange("(a p) n -> p a n", p=128))
            # xt token-columns 0:512 feed the ni=0 groups
            nc.scalar.dma_start(
                out=xtall[:, :].rearrange("p (a t) -> p a t", a=NK)
                [:, :, 0:512],
                in_=xT[:, 0:512].rearrange("(a p) t -> p a t", p=128))
            nc.sync.dma_start(
                out=wqkall[:, :].rearrange("p (a n) -> p a n", a=NK)
                [:, :, 0:256],
                in_=wqk[:, 0:256].rearrange("(a p) n -> p a n", p=128))
            nc.scalar.dma_start(
                out=xtall[:, :].rearrange("p (a t) -> p a t", a=NK)
                [:, :, 512:1024],
                in_=xT[:, 512:1024].rearrange("(a p) t -> p a t", p=128))
            wqk_sb = [wqkall[:, ki * 512:(ki + 1) * 512] for ki in range(NK)]
            xt_sb = [xtall[:, ki * T:(ki + 1) * T] for ki in range(NK)]
            wvall = big.tile([128, NK * HPC * D], F16, name="wvall")
            nc.sync.dma_start(
                out=wvall[:, :].rearrange("p (a n) -> p a n", a=NK),
                in_=wv[:, :].rearrange("(a p) n -> p a n", p=128))
            wv_sb = [wvall[:, ki * HPC * D:(ki + 1) * HPC * D]
                     for ki in range(NK)]
            wpall = big.tile([128, 2 * C], F16, name="wpall")
            nc.scalar.dma_start(
                out=wpall[:, :].rearrange("p (a n) -> p a n", a=2),
                in_=wp[:, :].rearrange("(a p) n -> p a n", p=128))
            wp_sb = [wpall[:, ci2 * C:(ci2 + 1) * C] for ci2 in range(2)]

            # ---- PE warm-up: keep TensorE busy while inputs stream in ----
            for wi in range(10):
                wps = ps.tile([128, 512], F32, name="pk", bufs=2)
                nc.tensor.matmul(wps[:, :], id_sb, c16[:, 0:512],
                                 start=True, stop=True)

            # ---- persistent intermediates ----
            qt_sb = [big.tile([128, T], F16, name=f"qt{j}") for j in range(2)]
            ktsq_sb = [big.tile([128, T], F16, name=f"ktsq{h}") for h in range(HPC)]
            eq_sb = [big.tile([128, T], F16, name=f"eq{h}") for h in range(HPC)]
            ekt_sb = [big.tile([128, T], F16, name=f"ekt{h}") for h in range(HPC)]
            v_sb = [big.tile([128, HPC * (D + 1)], F16, name=f"v{ti}")
                    for ti in range(NT)]
            yt_sb = [big.tile([128, T], F16, name=f"yt{j}") for j in range(2)]

            # ---- phases 1/1b/2, pipelined by T-halves ----
            def qk_group(mi, ni):
                tsl = slice(ni * 512, (ni + 1) * 512)
                p_ = bankA()
                for ki in range(NK):
                    nc.tensor.matmul(
                        p_[:, :],
                        wqk_sb[ki][:, mi * 128:(mi + 1) * 128],
                        xt_sb[ki][:, tsl],
                        start=(ki == 0), stop=(ki == NK - 1))
                if mi < 2:
                    nc.vector.tensor_scalar_add(
                        qt_sb[mi][:, tsl], p_[:, :], bqk_sb[mi])
                else:
                    for par in range(2):
                        h = (mi - 2) * 2 + par
                        rs = par * 64
                        nc.vector.tensor_scalar_add(
                            ktsq_sb[h][0:64, tsl], p_[rs:rs + 64, :],
                            bqk_sb[mi][rs:rs + 64, :])
                        nc.vector.tensor_mul(
                            ktsq_sb[h][64:128, tsl],
                            ktsq_sb[h][0:64, tsl],
                            ktsq_sb[h][0:64, tsl])

            def e_group(h, ni):
                mi, rs = h // 2, (h % 2) * 64
                tsl = slice(ni * 512, (ni + 1) * 512)
                pq = bankA()
                nc.tensor.matmul(pq[:, :], om_sb[rs:rs + 64, :],
                                 qt_sb[mi][rs:rs + 64, tsl],
                                 start=True, stop=True)
                nc.scalar.activation(eq_sb[h][:, tsl], pq[:, :], Exp,
                                     bias=lnsc_sb[:, :], scale=1.0)
                pk2 = bankA()
                nc.tensor.matmul(pk2[:, :], om_sb[0:64, :],
                                 ktsq_sb[h][0:64, tsl],
                                 start=True, stop=True)
                nc.scalar.activation(ekt_sb[h][:, tsl], pk2[:, :], Exp,
                                     bias=lnsc_sb[:, :], scale=1.0)

            def v_group(ti):
                nc.vector.memset(
                    v_sb[ti][:, :].rearrange("p (h c) -> p h c", c=D + 1)
                    [:, :, D:D + 1], 1.0)
                p_ = bankA()
                for ki in range(NK):
                    nc.tensor.matmul(
                        p_[:, 0:HPC * D],
                        xt_sb[ki][:, ti * 128:(ti + 1) * 128],
                        wv_sb[ki][:, :],
                        start=(ki == 0), stop=False)
                nc.tensor.matmul(p_[:, 0:HPC * D], ones_r[:, :], bv_sb[:, :],
                                 start=False, stop=True)
                nc.vector.tensor_copy(
                    v_sb[ti][:, :].rearrange("p (h c) -> p h c", c=D + 1)
                    [:, :, 0:D],
                    p_[:, 0:HPC * D].rearrange("p (h c) -> p h c", c=D))

            for mi in (2, 3, 0, 1):
                qk_group(mi, 0)
            for h in range(HPC):
                e_group(h, 0)
            for ti in range(4):
                v_group(ti)
            for mi in (2, 3, 0, 1):
                qk_group(mi, 1)
            for h in range(HPC):
                e_group(h, 1)
            for ti in range(4, NT):
                v_group(ti)

            # ---- phase 3: chunked FAVOR, pair-batched ----
            # Per head pair (h0=2p, h1=2p+1), per chunk: all per-head tiles
            # live side-by-side so exp/recip/normalize/transpose/snapshot run
            # as single batched ops. PSUM start=True only on the first write
            # of a bank; later writes rely on overwrite-where-bit-unset.
            for pair in range(2):
                h0, h1 = 2 * pair, 2 * pair + 1
                mi = pair
                s_pair = chk.tile([128, 2 * (D + 1)], F16, name="Sp")
                ps_s = ps.tile([128, 2 * (D + 1)], F32, name="psS", bufs=1)
                for ci in range(NT):
                    csl = slice(ci * L, (ci + 1) * L)
                    # [projk|-nsq] for both heads into one bank
                    pk = ps.tile([128, 258], F32, name="pk", bufs=2)
                    nc.tensor.matmul(pk[:, 0:129], ktsq_sb[h0][:, csl],
                                     on_sb[:, :], start=True, stop=True,
                                     skip_group_check=True)
                    nc.tensor.matmul(pk[:, 129:258], ktsq_sb[h1][:, csl],
                                     on_sb[:, :], start=False, stop=True,
                                     skip_group_check=True)
                    fk2 = col.tile([128, 2], F32, name="fk2")
                    nc.scalar.activation(
                        fk2,
                        pk[:, :].rearrange("p (a c) -> p a c", a=2)
                        [:, :, 128:129].rearrange("p a c -> p (a c)"),
                        Exp, bias=nhm_sb[:, :], scale=1.0)
                    ekh = chk.tile([128, 256], F16, name="ekh")
                    nc.scalar.activation(
                        ekh[:, :].rearrange("p (a c) -> p a c", a=2),
                        pk[:, :].rearrange("p (a c) -> p a c", a=2)
                        [:, :, 0:128],
                        Exp, bias=lnsc_sb[:, :], scale=1.0)
                    # A_T for both heads
                    pA = ps.tile([128, 256], F32, name="pA", bufs=1)
                    nc.tensor.matmul(pA[:, 0:128], ekt_sb[h0][:, csl],
                                     eq_sb[h0][:, csl], start=True, stop=True,
                                     skip_group_check=True)
                    nc.tensor.matmul(pA[:, 128:256], ekt_sb[h1][:, csl],
                                     eq_sb[h1][:, csl], start=False, stop=True,
                                     skip_group_check=True)
                    at0 = chk.tile([128, 256], F16, name="at0")
                    nc.scalar.activation(
                        at0[:, 0:128], pA[:, 0:128],
                        mybir.ActivationFunctionType.Copy,
                        bias=0.0, scale=fk2[:, 0:1])
                    nc.scalar.activation(
                        at0[:, 128:256], pA[:, 128:256],
                        mybir.ActivationFunctionType.Copy,
                        bias=0.0, scale=fk2[:, 1:2])
                    atm = chk.tile([128, 256], F16, name="atm")
                    mk_b = bass.AP(
                        tensor=mk_sb.tensor, offset=mk_sb.offset,
                        ap=[mk_sb.ap[0], [0, 2], mk_sb.ap[1]])
                    nc.vector.tensor_tensor(
                        atm[:, :].rearrange("p (a c) -> p a c", a=2),
                        at0[:, :].rearrange("p (a c) -> p a c", a=2),
                        mk_b, op=mybir.AluOpType.mult)
                    # fk-scaled V for both heads (state-update rhs)
                    vh = chk.tile([128, 2 * (D + 1)], F16, name="vh")
                    nc.vector.tensor_scalar_mul(
                        vh[:, 0:D + 1],
                        v_sb[ci][:, h0 * (D + 1):(h0 + 1) * (D + 1)],
                        fk2[:, 0:1])
                    nc.vector.tensor_scalar_mul(
                        vh[:, D + 1:],
                        v_sb[ci][:, h1 * (D + 1):(h1 + 1) * (D + 1)],
                        fk2[:, 1:2])
                    # num/den for both heads: [num0|den0|num1|den1]
                    pY = ps.tile([128, 2 * (D + 1)], F32, name="pY", bufs=1)
                    for idx, h in enumerate((h0, h1)):
                        ysl = slice(idx * (D + 1), (idx + 1) * (D + 1))
                        if ci > 0:
                            nc.tensor.matmul(
                                pY[:, ysl], eq_sb[h][:, csl],
                                s_pair[:, ysl],
                                start=(idx == 0), stop=True,
                                skip_group_check=True)
                        nc.tensor.matmul(
                            pY[:, ysl],
                            atm[:, idx * 128:(idx + 1) * 128],
                            v_sb[ci][:, h * (D + 1):(h + 1) * (D + 1)],
                            start=(ci == 0 and idx == 0), stop=True,
                            skip_group_check=True)
                    # y = num/den, both heads in one go
                    rc2 = col.tile([128, 2], F32, name="rc2")
                    nc.vector.reciprocal(
                        rc2,
                        pY[:, :].rearrange("p (a c) -> p a c", a=2)
                        [:, :, D:D + 1].rearrange("p a c -> p (a c)"))
                    ych = chk.tile([128, 128], F16, name="ych")
                    rc_b = bass.AP(
                        tensor=rc2.tensor, offset=rc2.offset,
                        ap=[rc2.ap[0], rc2.ap[1], [0, D]])
                    nc.vector.tensor_tensor(
                        ych[:, :].rearrange("p (a c) -> p a c", a=2),
                        pY[:, :].rearrange("p (a c) -> p a c", a=2)[:, :, 0:D],
                        rc_b, op=mybir.AluOpType.mult)
                    # yT for both heads via one PE transpose
                    pyt = ps.tile([128, 128], F16, name="pyt", bufs=1)
                    nc.tensor.transpose(pyt[:, :], ych[:, :], id_sb[:, :])
                    nc.vector.tensor_copy(yt_sb[mi][:, csl], pyt[:, :])
                    # state update for both heads
                    nc.tensor.matmul(ps_s[:, 0:D + 1], ekh[:, 0:128],
                                     vh[:, 0:D + 1],
                                     start=(ci == 0), stop=(ci == NT - 1),
                                     skip_group_check=True)
                    nc.tensor.matmul(ps_s[:, D + 1:], ekh[:, 128:256],
                                     vh[:, D + 1:],
                                     start=False, stop=(ci == NT - 1),
                                     skip_group_check=True)
                    if ci < NT - 1:
                        nc.vector.tensor_copy(s_pair[:, :], ps_s[:, :])

            # ---- phase 4: output projection ----
            for ti in range(NT):
                osb = cpy.tile([128, 1024], F32, name="osb")
                for ni in range(2):
                    nsl = slice(ni * 512, (ni + 1) * 512)
                    pp = bankA()
                    for ci2 in range(2):
                        nc.tensor.matmul(pp[:, :],
                                         yt_sb[ci2][:, ti * 128:(ti + 1) * 128],
                                         wp_sb[ci2][:, nsl],
                                         start=(ci2 == 0), stop=(ci2 == 1))
                    if (2 * ti + ni) % 8 < 3:
                        nc.scalar.copy(osb[:, nsl], pp[:, :])
                    else:
                        nc.vector.tensor_copy(osb[:, nsl], pp[:, :])
                (nc.sync if ti % 2 == 0 else nc.scalar).dma_start(
                    out=outp[ti * 128:(ti + 1) * 128, :], in_=osb[:, :])

    _split_waits(nc)
    return nc


_NC_CACHE = None


def _get_nc():
    global _NC_CACHE
    if _NC_CACHE is None:
        _NC_CACHE = build_bass()
    return _NC_CACHE


def kernel(x, W_attn, b_attn, W_proj, b_proj, omega):
    from concourse.bass_utils import run_bass_kernel_spmd

    x = np.asarray(x, dtype=np.float32)
    W_attn = np.asarray(W_attn, dtype=np.float32)
    b_attn = np.asarray(b_attn, dtype=np.float32)
    W_proj = np.asarray(W_proj, dtype=np.float32)
    b_proj = np.asarray(b_proj, dtype=np.float32)
    omega = np.asarray(omega, dtype=np.float32)

    B = x.shape[0]
    scale = 1.0 / math.sqrt(D)
    omega2 = np.concatenate([omega, omega], axis=0).astype(np.float16)
    omnsq = np.zeros((128, 129), np.float32)
    omnsq[0:64, 0:128] = omega
    omnsq[64:128, 128] = -0.5
    omnsq = omnsq.astype(np.float16)
    maskT = np.triu(np.ones((128, 128), np.float32)).astype(np.float16)
    ident = np.eye(128, dtype=np.float16)
    c16_base = np.zeros((128, 769), np.float16)
    c16_base[:, 0:128] = omega2
    c16_base[:, 128:257] = omnsq
    c16_base[:, 257:385] = maskT
    c16_base[:, 385:513] = ident
    xTs = [np.ascontiguousarray(x[b].T).astype(np.float16) for b in range(B)]

    in_maps = []
    for core in range(8):
        b, g = core // 4, core % 4
        ch0 = g * HPC * D
        wq = W_attn[:, ch0:ch0 + HPC * D] * scale
        wk = W_attn[:, C + ch0:C + ch0 + HPC * D] * scale
        wqk_ = np.concatenate([wq, wk], axis=1).astype(np.float16)
        wv_ = W_attn[:, 2 * C + ch0:2 * C + ch0 + HPC * D].astype(np.float16)
        bqk_ = (np.concatenate([b_attn[ch0:ch0 + HPC * D],
                                b_attn[C + ch0:C + ch0 + HPC * D]]) * scale
                ).astype(np.float32).reshape(-1, 1)
        bv_ = b_attn[2 * C + ch0:2 * C + ch0 + HPC * D].astype(
            np.float16).reshape(1, -1)
        wp_ = W_proj[ch0:ch0 + HPC * D, :].astype(np.float16)
        c16 = c16_base.copy()
        c16[0, 513:513 + HPC * D] = bv_[0]
        c32 = bqk_.reshape(4, 128).T.astype(np.float32)
        in_maps.append({
            "xT": xTs[b], "wqk": wqk_, "wv": wv_, "wp": wp_,
            "consts16": c16, "consts32": np.ascontiguousarray(c32),
        })

    nc = _get_nc()
    res = run_bass_kernel_spmd(nc, in_maps, list(range(8)))

    out = np.zeros((B, T, C), dtype=np.float32)
    for core in range(8):
        out[core // 4] += res.results[core]["outp"]
    out += b_proj[None, None, :]
    return out
